# revision 44
# baseline (speedup 1.0000x reference)
"""Multi-head attention forward for TRN2, 8 NeuronCores, data-parallel over batch.

Reference computation (B=16, S=1024, D=768, H=12, HD=64), fp32:
    q = einsum('bsd,dhe->bshe', x, Wq) + bq        (same for k, v)
    z = einsum('bqhd,bkhd->bhqk', q/8, k)
    a = softmax(z, axis=-1)
    o = einsum('bhqk,bkhd->bqhd', a, v)
    y = einsum('bqhd,hde->bqe', o, Wo) + bo

Design (per core, 2 batches, phases pipelined by the Tile scheduler):
  - x [S,D] -> xT [D,S] via PE fp32 transposes, evicted to bf16.
  - Q/K/V projections in bf16 (natural layout, chunked contraction).
  - Q/K evictions scale by 64 and round to fp8e4 in the natural layout,
    then 24 small gpsimd DMAs shuffle them into the DoubleRow quad layout
    QT_dr/KT_dr [P, 4, 2, S]: head h=3g+q at partitions 32q:32q+32, group
    g, plane pl <-> d=2r+pl. (Bases are restricted to {0,32,64} - PE
    quadrant 3 is unusable - so 3 heads per group, partitions 96:128
    idle.) fp8 q/k is the one lossy step: its error enters pre-softmax
    and is damped by the small score variance (~1.4e-2 of the 2e-2
    budget); everything else stays bf16-exact.
  - Scores as fp8 DoubleRow (0.5 cycles/row, 2 d-planes per instruction):
    zT[k,q] per head from KT_dr/QT_dr [32,2,*] slices; head pairs at
    disjoint 32-row bases so real hardware row-tiles them concurrently.
  - exp on ACT with scale=(1/8)/4096 fused (absorbs the 64*64 q/k
    prescale), output bf16 at [k, q] tiles - 192 exps of [128,1024] are
    the ACT floor (~200us) that bounds the kernel.
  - PV with at as the STATIONARY operand: per (head, q-tile), 8
    accumulating matmuls lhsT=at[:,qt*128:+128], rhs=V[:,kt,h,0:65]
    (moving free dim = 65, so the whole PV pass is ~50k cycles). V
    carries a ones column, so the softmax denominator lands per-partition
    in PSUM column 64 - the normalization becomes one tiny reciprocal
    [128,8] plus one broadcast-AP multiply per head. No partition
    broadcasts, no gpsimd shifts.
  - O [q, hd] transposes back to OTn [hd, q] via bf16 PE transposes (odd
    heads straight to partitions 64:128), then the output projection runs
    in bf16 over chunk pairs exactly like the baseline.
  - Weight loads/conversions hoisted out of the batch loop.
"""

import numpy as np
from contextlib import ExitStack

import concourse.bacc as bacc
import concourse.bass as bass
import concourse.tile as tile
import concourse.mybir as mybir
from concourse.bass_utils import run_bass_kernel_spmd
from concourse.masks import make_identity

B, S, D, H, HD = 16, 1024, 768, 12, 64
NCORES = 8
BL = B // NCORES      # batches per core
P = 128
DC = D // P           # 6 contraction chunks
SQ = S // P           # 8 seq tiles of 128
F32 = mybir.dt.float32
F32R = mybir.dt.float32r
BF16 = mybir.dt.bfloat16
F8 = mybir.dt.float8e4
DR = mybir.MatmulPerfMode.DoubleRow
EXP = mybir.ActivationFunctionType.Exp
SCALE = 1.0 / float(np.sqrt(HD))
QS = 64.0                      # q/k fp8 pre-scale
ESCALE = SCALE / (QS * QS)     # exp scale absorbing the q/k scaling

_NC = {}
_DEBUG = False  # add DRAM dumps of intermediates (batch 0)


def _emit(tc, x_d, w_d, b_d, y_d, dbg=None, with_bias=True):
    """Emit the whole per-core program. w_d/b_d: dicts of DRAM APs."""
    nc = tc.nc

    def dump(name, sbuf_ap):
        if dbg is not None and name in dbg:
            nc.sync.dma_start(out=dbg[name], in_=sbuf_ap)

    with ExitStack() as ctx:
        consts = ctx.enter_context(tc.tile_pool(name="consts", bufs=1))
        wpool = ctx.enter_context(tc.tile_pool(name="wpool", bufs=1))
        big = ctx.enter_context(tc.tile_pool(name="big", bufs=2))
        atp = ctx.enter_context(tc.tile_pool(name="atp", bufs=16))
        iop = ctx.enter_context(tc.tile_pool(name="iop", bufs=2))
        smal = ctx.enter_context(tc.tile_pool(name="smal", bufs=4))
        pp = ctx.enter_context(tc.tile_pool(name="pp", bufs=2, space="PSUM"))

        # ---- constants ----
        ident = consts.tile([P, P], F32)
        make_identity(nc, ident)
        identb = consts.tile([P, P], BF16)
        nc.vector.tensor_copy(identb, ident)
        if with_bias:
            bq_sb = consts.tile([P, DC], F32)
            nc.sync.dma_start(out=bq_sb,
                              in_=b_d["bq"].rearrange("(c p) -> p c", p=P))
            nc.vector.tensor_scalar_mul(bq_sb, bq_sb, QS)
            bk_sb = consts.tile([P, DC], F32)
            nc.sync.dma_start(out=bk_sb,
                              in_=b_d["bk"].rearrange("(c p) -> p c", p=P))
            nc.vector.tensor_scalar_mul(bk_sb, bk_sb, QS)
            bv_st = consts.tile([P, DC], F32)
            nc.sync.dma_start(out=bv_st,
                              in_=b_d["bv"].rearrange("(c p) -> p c", p=P))
            bv_r = consts.tile([P, DC], BF16)
            nc.vector.tensor_copy(bv_r, bv_st)
            bo_st = consts.tile([1, D], F32)
            nc.sync.dma_start(out=bo_st, in_=b_d["bo"].unsqueeze(0))
            bo_r = consts.tile([1, D], BF16)
            nc.vector.tensor_copy(bo_r, bo_st)
            ones_f32 = consts.tile([1, P], F32)
            nc.vector.memset(ones_f32, 1.0)
            ones_row_r = consts.tile([1, P], BF16)
            nc.vector.tensor_copy(ones_row_r, ones_f32)
            cvec_sb = consts.tile([1, D], BF16)
        # warm the ACT exp table at t=0 so the ~1.3us table load overlaps
        # phase A instead of stalling the first attention tile
        expwarm = consts.tile([1, 1], F32)
        nc.scalar.activation(expwarm, ident[0:1, 0:1], EXP)
        cvec_state = {"done": False}

        # ---- weights: load once, convert to bf16 ----
        def load_weight(name):
            wr = wpool.tile([P, DC, D], BF16, tag=f"w_{name}", name=f"w_{name}")
            src = w_d[name].rearrange("(c p) m -> p c m", p=P)
            for c in range(0, DC, 2):
                ws = iop.tile([P, 2, D], F32, tag="st2", name=f"ws_{name}_{c}")
                nc.sync.dma_start(out=ws, in_=src[:, c:c + 2, :])
                nc.vector.tensor_copy(wr[:, c:c + 2, :], ws)
            return wr

        weights = {}

        def get_weight(name):
            if name not in weights:
                weights[name] = load_weight(name)
            return weights[name]

        def emit_cvec():
            wo_r = get_weight("wo")
            cv = pp.tile([P, 512], F32, tag="mm", name="cvps")
            cv2 = pp.tile([P, 256], F32, tag="mm", name="cvps2")
            for c in range(DC):
                nc.tensor.matmul(cv[0:1, :], bv_r[:, c:c + 1],
                                 wo_r[:, c, 0:512], start=(c == 0),
                                 stop=False)
                nc.tensor.matmul(cv2[0:1, :], bv_r[:, c:c + 1],
                                 wo_r[:, c, 512:D], start=(c == 0),
                                 stop=False)
            nc.tensor.matmul(cv[0:1, :], ones_row_r[:, 0:1],
                             bo_r[:, 0:512], start=False, stop=True)
            nc.tensor.matmul(cv2[0:1, :], ones_row_r[:, 0:1],
                             bo_r[:, 512:D], start=False, stop=True)
            nc.vector.tensor_copy(cvec_sb[:, 0:512], cv[0:1, :])
            nc.vector.tensor_copy(cvec_sb[:, 512:D], cv2[0:1, :])

        def phase_D(b, OTn, y_b):
            ctx_d = tc.high_priority()
            ctx_d.__enter__()
            wo_r = get_weight("wo")
            if with_bias and not cvec_state["done"]:
                cvec_state["done"] = True
                emit_cvec()
            for sq in range(0, SQ, 2):
                yst = iop.tile([P, 2, D], F32, tag="st2", name=f"yst_{b}_{sq}")
                # split the final store so its first half ships while the
                # last tile is still evicting (shorter kernel tail)
                split = (b == BL - 1 and sq == SQ - 2)
                for j in range(2):
                    y0 = pp.tile([P, 512], F32, tag="mm",
                                 name=f"y0_{b}_{sq}_{j}")
                    y1 = pp.tile([P, 256], F32, tag="mm",
                                 name=f"y1_{b}_{sq}_{j}")
                    for c in range(DC):
                        st = OTn[:, c, (sq + j) * P:(sq + j + 1) * P]
                        last = (not with_bias) and c == DC - 1
                        nc.tensor.matmul(y0, st, wo_r[:, c, 0:512],
                                         start=(c == 0), stop=last)
                        nc.tensor.matmul(y1, st, wo_r[:, c, 512:D],
                                         start=(c == 0), stop=last)
                    if with_bias:
                        nc.tensor.matmul(y0, ones_row_r, cvec_sb[:, 0:512],
                                         start=False, stop=True)
                        nc.tensor.matmul(y1, ones_row_r, cvec_sb[:, 512:D],
                                         start=False, stop=True)
                    nc.vector.tensor_copy(yst[:, j, 0:512], y0)
                    if split and j == 1:
                        nc.sync.dma_start(out=y_b[:, sq + j, 0:512],
                                          in_=yst[:, j, 0:512])
                    nc.vector.tensor_copy(yst[:, j, 512:D], y1)
                    if split and j == 1:
                        nc.sync.dma_start(out=y_b[:, sq + j, 512:D],
                                          in_=yst[:, j, 512:D])
                    elif split:
                        nc.sync.dma_start(out=y_b[:, sq + j, :],
                                          in_=yst[:, j, :])
                if not split:
                    nc.sync.dma_start(out=y_b[:, sq:sq + 2, :], in_=yst)
            ctx_d.__exit__(None, None, None)

        state = {}
        for b in range(BL):
            x_b = x_d[b].rearrange("(t p) d -> p t d", p=P)
            y_b = y_d[b].rearrange("(t p) d -> p t d", p=P)

            # ---- phase A: x -> xT (bf16) ----
            # gpsimd converts x to bf16 so the PE transposes run at 1
            # cycle/row instead of fp32's 2 (and evictions get DVE 2x mode)
            xT = big.tile([P, DC, S], BF16, tag="xT", name=f"xT_{b}")
            for sq in range(0, SQ, 2):
                x_in = iop.tile([P, 2, D], F32, tag="st2", name=f"xin_{b}_{sq}")
                if b == 0 and sq == 0:
                    # split the first load so the first transposes start as
                    # soon as the first columns land (shorter kernel lead-in)
                    nc.sync.dma_start(out=x_in[:, 0, 0:384],
                                      in_=x_b[:, sq, 0:384])
                    nc.sync.dma_start(out=x_in[:, 0, 384:D],
                                      in_=x_b[:, sq, 384:D])
                    nc.sync.dma_start(out=x_in[:, 1, :], in_=x_b[:, sq + 1, :])
                else:
                    nc.sync.dma_start(out=x_in, in_=x_b[:, sq:sq + 2, :])
                xb = iop.tile([P, 2, D], BF16, tag="xb", name=f"xb_{b}_{sq}",
                              bufs=2)
                nc.gpsimd.tensor_copy(xb, x_in)
                for j in range(2):
                    scol = slice((sq + j) * P, (sq + j + 1) * P)
                    tt0 = pp.tile([P, 512], BF16, tag="mm",
                                  name=f"t0_{b}_{sq}_{j}")
                    for c in range(4):
                        nc.tensor.transpose(
                            tt0[:, c * P:(c + 1) * P],
                            xb[:, j, c * P:(c + 1) * P], identb)
                    nc.vector.tensor_copy(
                        xT[:, 0:4, scol],
                        tt0.rearrange("p (c q) -> p c q", c=4))
                    tt1 = pp.tile([P, 256], BF16, tag="mm",
                                  name=f"t1_{b}_{sq}_{j}")
                    for c in range(2):
                        nc.tensor.transpose(
                            tt1[:, c * P:(c + 1) * P],
                            xb[:, j, (4 + c) * P:(5 + c) * P], identb)
                    nc.vector.tensor_copy(
                        xT[:, 4:6, scol],
                        tt1.rearrange("p (c q) -> p c q", c=2))

            if b == 0:
                dump("xT", xT)

            # ---- phase B: projections (bf16) ----
            # Q/K: accumulate per natural m-block/half, evict *64 to fp8
            # staging, then shuffle-DMA into the quad layout.
            QT = big.tile([P, 4, 2, S], F8, tag="QT", name=f"QT_{b}")
            KT = big.tile([P, 4, 2, S], F8, tag="KT", name=f"KT_{b}")
            stgs = {"q": big.tile([P, DC, S], F8, tag="qstg",
                                  name=f"qstg_{b}", bufs=1),
                    "k": big.tile([P, DC, S], F8, tag="kstg",
                                  name=f"kstg_{b}", bufs=1)}
            def emit_qk(mb):
                for (wr, bname, qs, qdr) in (
                        (get_weight("wq"), "bq", "q", QT),
                        (get_weight("wk"), "bk", "k", KT)):
                    stg = stgs[qs]
                    for hf in range(2):
                        qq = pp.tile([P, 512], F32, tag="mm",
                                     name=f"{qs}ps_{b}_{mb}_{hf}")
                        for c in range(DC):
                            nc.tensor.matmul(
                                qq, wr[:, c, mb * P:(mb + 1) * P],
                                xT[:, c, hf * 512:(hf + 1) * 512],
                                start=(c == 0), stop=(c == DC - 1))
                        dst = stg[:, mb, hf * 512:(hf + 1) * 512]
                        if with_bias:
                            bias = bq_sb if bname == "bq" else bk_sb
                            nc.vector.tensor_scalar(
                                dst, qq, QS, bias[:, mb:mb + 1],
                                mybir.AluOpType.mult, mybir.AluOpType.add)
                        else:
                            nc.vector.tensor_scalar_mul(dst, qq, QS)
                    # shuffle natural block mb (heads 2mb,2mb+1; row 64j+d)
                    # into quad: head h at partitions 32*(h%3)+r, group
                    # h//3, plane pl=d%2, with d=2r+pl.
                    for j in range(2):
                        h = 2 * mb + j
                        g, qb = h // 3, 32 * (h % 3)
                        dsl = stg[64 * j:64 * (j + 1), mb, :]
                        # one DMA per plane: a partition-crossing middle
                        # dim ([pitch, 2]) is silently dropped by the DGE,
                        # but a strided partition dim ([2*pitch, 32]) works
                        for pl in range(2):
                            src = bass.AP(
                                tensor=dsl.tensor,
                                offset=dsl.offset + pl * dsl.ap[0][0],
                                ap=[[2 * dsl.ap[0][0], 32],
                                    list(dsl.ap[1])])
                            nc.sync.dma_start(
                                out=qdr[qb:qb + 32, g, pl, :], in_=src)

            # V layout [P, SQ, H, 65] bf16: cols 0..63 = v, col 64 = ones
            V = big.tile([P, SQ, H, 65], BF16, tag="V", name=f"V_{b}")
            nc.vector.memset(V[:, :, :, 64], 1.0)

            def emit_v(sq):
                wv_r = get_weight("wv")
                vv0 = pp.tile([P, 512], F32, tag="mm", name=f"v0_{b}_{sq}")
                vv1 = pp.tile([P, 256], F32, tag="mm", name=f"v1_{b}_{sq}")
                for c in range(DC):
                    nc.tensor.matmul(
                        vv0, xT[:, c, sq * P:(sq + 1) * P],
                        wv_r[:, c, 0:512], start=(c == 0), stop=(c == DC - 1))
                    nc.tensor.matmul(
                        vv1, xT[:, c, sq * P:(sq + 1) * P],
                        wv_r[:, c, 512:D], start=(c == 0), stop=(c == DC - 1))
                nc.vector.tensor_copy(
                    V[:, sq, 0:8, 0:64],
                    vv0.rearrange("p (h e) -> p h e", h=8))
                nc.vector.tensor_copy(
                    V[:, sq, 8:12, 0:64],
                    vv1.rearrange("p (h e) -> p h e", h=4))

            # interleave so qk block k lands before head 2k's scores and
            # V(sq) stays ahead of the PV consuming kt=sq
            emit_v(0)
            emit_v(1)
            emit_qk(0)
            emit_v(2)
            emit_v(3)
            emit_qk(1)
            emit_v(4)
            emit_v(5)
            emit_qk(2)
            emit_v(6)
            emit_v(7)
            emit_qk(3)
            emit_qk(4)
            emit_qk(5)
            if b == 0:
                dump("QT", QT)
                dump("KT", KT)
                dump("V", V)

            state[b] = (QT, KT, V, xT, y_b)

        for b in range(BL):
            QT, KT, V, xT, y_b = state[b]
            # ---- phase C: attention, one head per iteration ----
            # scores: fp8 DoubleRow. PV: at-stationary, V-moving, all 8
            # q-tiles accumulate per kt into one [P, 8, 128]-strided psum
            # tile (2 banks; bank-aligned groups -> two start=True marks,
            # qt0 and qt4, lazily zero exactly one bank each). The softmax
            # denominator lands in column 64 of each q-tile group.
            OTn = big.tile([P, DC, S], BF16, tag="OTn", name=f"OTn_{b}",
                           bufs=1)
            ctx_c = tc.high_priority()
            ctx_c.__enter__()
            for h in range(H):
                g, base = h // 3, 32 * (h % 3)
                oos = pp.tile([P, SQ, P], F32, tag="ov",
                              name=f"oo_{b}_{h}", bufs=1)
                for kt in range(SQ):
                    zz = pp.tile([P, 1024], F32, tag="zz",
                                 name=f"zps_{b}_{h}_{kt}")
                    for hf in range(2):
                        ksl = KT[base:base + 32, g, :, kt * P:(kt + 1) * P]
                        qsl = QT[base:base + 32, g, :,
                                 hf * 512:(hf + 1) * 512]
                        nc.tensor.matmul(
                            zz[:, hf * 512:(hf + 1) * 512],
                            ksl, qsl, start=True, stop=True, perf_mode=DR)
                    at = atp.tile([P, S], BF16, tag="at",
                                  name=f"at_{b}_{h}_{kt}")
                    nc.scalar.activation(at, zz, EXP, scale=ESCALE)
                    if b == 0 and h == 0 and kt == 0:
                        dump("at0", at)
                    vsl = V[:, kt, h, :]
                    for qt in range(SQ):
                        nc.tensor.matmul(
                            oos[:, qt, 0:65],
                            at[:, qt * P:(qt + 1) * P], vsl,
                            start=(kt == 0 and qt % 4 == 0),
                            stop=(kt == SQ - 1),
                            skip_group_check=True)
                # normalize: rb = 1/denom (col 64 of each group),
                # per-partition broadcast multiply
                rb = smal.tile([P, SQ], F32, tag="rb", name=f"rb_{b}_{h}")
                nc.vector.reciprocal_approx_fast(out=rb, in_=oos[:, :, 64])
                osb = smal.tile([P, SQ, HD], BF16, tag="osb",
                                name=f"osb_{b}_{h}")
                rbb = bass.AP(tensor=rb.tensor, offset=rb.offset,
                              ap=[list(rb.ap[0]), list(rb.ap[1]), [0, HD]])
                nc.vector.tensor_mul(osb, oos[:, :, 0:64], rbb)
                if b == 0 and h == 0:
                    dump("rbi0", rb)
                    dump("osb0", osb)
                # transpose O [q, hd] -> OTn [hd, q] (odd heads straight to
                # rows 64:128)
                j = h % 2
                tps = pp.tile([P, 1024], BF16, tag="ov",
                              name=f"tops_{b}_{h}", bufs=1)
                for qt in range(SQ):
                    nc.tensor.transpose(
                        tps[64 * j:64 * (j + 1), qt * P:(qt + 1) * P],
                        osb[:, qt, :], identb)
                nc.vector.tensor_copy(OTn[64 * j:64 * (j + 1), h // 2, :],
                                      tps[64 * j:64 * (j + 1), :])
            ctx_c.__exit__(None, None, None)
            if b == 0:
                dump("OTn", OTn)

            phase_D(b, OTn, y_b)


def _build(with_bias=True):
    nc = bacc.Bacc("TRN2", target_bir_lowering=False, debug=False,
                   num_devices=NCORES)
    x_d = nc.dram_tensor("x", [BL, S, D], F32, kind="ExternalInput").ap()
    w_d = {n: nc.dram_tensor(n, [D, D], F32, kind="ExternalInput").ap()
           for n in ("wq", "wk", "wv", "wo")}
    b_d = {n: nc.dram_tensor(n, [D], F32, kind="ExternalInput").ap()
           for n in ("bq", "bk", "bv", "bo")}
    y_d = nc.dram_tensor("y", [BL, S, D], F32, kind="ExternalOutput").ap()
    dbg = None
    if _DEBUG:
        shapes = {"xT": ([P, DC, S], BF16), "QT": ([P, 4, 2, S], F8),
                  "KT": ([P, 4, 2, S], F8), "V": ([P, SQ, H, 65], BF16),
                  "at0": ([P, S], BF16), "rbi0": ([P, 4], F32),
                  "osb0": ([P, SQ, HD], BF16), "OTn": ([P, DC, S], BF16)}
        dbg = {n: nc.dram_tensor(f"dbg_{n}", sh, dt,
                                 kind="ExternalOutput").ap()
               for n, (sh, dt) in shapes.items()}
    with tile.TileContext(nc) as tc:
        _emit(tc, x_d, w_d, b_d, y_d, dbg, with_bias=with_bias)
    nc.compile()
    return nc


def _in_maps(x, Wq, bq, Wk, bk, Wv, bv, Wo, bo):
    # convert to host numpy before reshaping so jax-array inputs don't
    # trigger device-side ops
    def _np(a, shape):
        return np.ascontiguousarray(
            np.asarray(a, dtype=np.float32).reshape(shape))

    w = {
        "wq": _np(Wq, (D, D)), "wk": _np(Wk, (D, D)),
        "wv": _np(Wv, (D, D)), "wo": _np(Wo, (D, D)),
        "bq": _np(bq, (D,)), "bk": _np(bk, (D,)),
        "bv": _np(bv, (D,)), "bo": _np(bo, (D,)),
    }
    x = np.asarray(x, dtype=np.float32)
    return [dict(w, x=np.ascontiguousarray(x[i * BL:(i + 1) * BL]))
            for i in range(NCORES)]


def get_nc(with_bias=True):
    if with_bias not in _NC:
        _NC[with_bias] = _build(with_bias=with_bias)
    return _NC[with_bias]


def run(inputs, trace=False):
    with_bias = any(
        np.any(np.asarray(inputs[k])) for k in ("bq", "bk", "bv", "bo"))
    nc = get_nc(with_bias=with_bias)
    maps = _in_maps(**inputs)
    res = run_bass_kernel_spmd(nc, maps, list(range(NCORES)), trace=trace)
    y = np.concatenate([res.results[i]["y"] for i in range(NCORES)], axis=0)
    return y, res


def kernel(x, Wq, bq, Wk, bk, Wv, bv, Wo, bo):
    y, _ = run(dict(x=x, Wq=Wq, bq=bq, Wk=Wk, bk=bk, Wv=Wv, bv=bv,
                    Wo=Wo, bo=bo))
    return y
